# revision 41
# baseline (speedup 1.0000x reference)
"""Multi-head attention (N=2, L=2048, 16 heads x 64) on 8 TRN2 NeuronCores.

Head-parallel attention (2 heads/core) with a per-batch software pipeline:
attention emission interleaved between projection blocks, per-batch AllToAll
(head-split -> sequence-split) hidden under compute, output projection
spread across the stream.

Scheduling (v2):
- PE p-state warmup: ~44 dependency-free matmuls issued at kernel start so
  the tensor engine ramps to full clock during the unavoidable launch+DMA
  window; real matmuls then start hot.
- Inputs split across four HWDGE queues (sync=y, vector=x, scalar=weights,
  gpsimd=spill+outputs); the first projection block's tiles arrive as
  128KB quarters so the first real matmul fires as soon as possible.
- Deferred softmax finish: the den-copy -> broadcast-matmul -> reciprocal
  -> normalize chain of attention block N is emitted inside block N+1's
  matmul stream, removing the per-block PE stall (and its p-state reset).
- Output projection chunks emitted mid-stream right after their collective
  lands; only the last 128-col chunk trails the final collective.
- Softmax normalization without DMA round trips: ones-column in V gives the
  denominator as AV row 64; K=1 matmul broadcasts it,
  reciprocal_approx_fast + one DVE multiply normalize.
- exp is one ScalarE call per k-tile covering both heads via a 3D AP.
"""
import sys

sys.path.insert(0, "/opt/trn_rl_repo")

import numpy as np
import ml_dtypes

import concourse.bass as bass
import concourse.bacc as bacc
import concourse.mybir as mybir
import concourse.tile as tile
from concourse.bass_utils import run_bass_kernel_spmd

BF16 = ml_dtypes.bfloat16

DM = 1024      # dmodel
DK = 64        # head dim
H = 16         # heads
NB = 2         # batch
L = 2048       # seq len
R = NB * L
NC = 8         # cores
HPC = H // NC  # heads per core = 2
DPC = HPC * DK  # depth per core = 128

SW = 512       # q sub-window
KT = 128       # k tile
NQS = L // SW   # 4 q blocks per batch
NKT = L // KT   # 16 k tiles per batch
CB = L // NC    # 256: per-batch per-core output chunk
VW = 65 * HPC   # 130: augmented v width (both heads, +ones col each)

N_WARM = 48    # p-state warmup matmuls (ap=512 each)

_CACHE = {}


def _classify_blocks(mask):
    """Per (qs, kt) block: 0=skip, 1=full, 2=partial (+ q-span, pattern)."""
    mask = np.asarray(mask, dtype=bool)
    cls = [[0] * NKT for _ in range(NQS)]
    span = [[None] * NKT for _ in range(NQS)]
    pat_ids = {}
    pats = []
    pat_idx = [[-1] * NKT for _ in range(NQS)]
    for qs in range(NQS):
        for kt in range(NKT):
            sub = mask[qs * SW:(qs + 1) * SW, kt * KT:(kt + 1) * KT]
            rows = np.nonzero(sub.any(axis=1))[0]
            if rows.size == 0:
                cls[qs][kt] = 0
            elif sub.all():
                cls[qs][kt] = 1
                span[qs][kt] = (0, SW)
            else:
                cls[qs][kt] = 2
                span[qs][kt] = (int(rows[0]), int(rows[-1]) + 1)
                pat = np.ascontiguousarray(sub.T).astype(BF16)  # [128 k, SW q]
                key = pat.tobytes()
                if key not in pat_ids:
                    pat_ids[key] = len(pats)
                    pats.append(pat)
                pat_idx[qs][kt] = pat_ids[key]
    # the first included kt of each sub-window must cover the full 512
    # columns (its start=True matmul clears PSUM has_written)
    for qs in range(NQS):
        for kt in range(NKT):
            if cls[qs][kt]:
                span[qs][kt] = (0, SW)
                break
    if not pats:
        pats.append(np.ones((KT, SW), dtype=BF16))
    return cls, span, pat_idx, np.stack(pats)


def _build(cls_, span_, pidx, n_pat):
    nc = bacc.Bacc("TRN2", target_bir_lowering=False, debug=False,
                   enable_asserts=False, num_devices=NC)
    f32, bf16 = mybir.dt.float32, mybir.dt.bfloat16
    EXP = mybir.ActivationFunctionType.Exp
    MUL = mybir.AluOpType.mult

    # weights arrive host-pre-shuffled into partition-major layouts so every
    # const DMA is one contiguous chunk per partition (few, large descriptors)
    xtb = nc.dram_tensor("xtb", [DM, R], bf16, kind="ExternalInput")
    ytb = nc.dram_tensor("ytb", [DM, R], bf16, kind="ExternalInput")
    wq = nc.dram_tensor("wq", [128, 8, DPC], bf16, kind="ExternalInput")
    wk = nc.dram_tensor("wk", [128, 8, DPC], bf16, kind="ExternalInput")
    wv = nc.dram_tensor("wv", [128, 8, VW], bf16, kind="ExternalInput")
    wo = nc.dram_tensor("wo", [128, 8, DM], bf16, kind="ExternalInput")
    bqd = nc.dram_tensor("bq", [DPC, 1], f32, kind="ExternalInput")
    bkd = nc.dram_tensor("bk", [DPC, 1], f32, kind="ExternalInput")
    bv1 = nc.dram_tensor("bv1", [1, VW], bf16, kind="ExternalInput")
    bod = nc.dram_tensor("bo", [128, 8, 1], f32, kind="ExternalInput")
    mpat = nc.dram_tensor("mpat", [KT, n_pat, SW], bf16, kind="ExternalInput")
    out_t = nc.dram_tensor("out_t", [DM, NB * CB], f32, kind="ExternalOutput")

    with tile.TileContext(nc) as tc:
        with (
            tc.tile_pool(name="cst", bufs=1) as cst,
            tc.tile_pool(name="xyq", bufs=32) as xyq,
            tc.tile_pool(name="xy", bufs=6) as xy,
            tc.tile_pool(name="big", bufs=1) as big,
            tc.tile_pool(name="expp", bufs=12) as expp,
            tc.tile_pool(name="nrm", bufs=3) as nrm,
            tc.tile_pool(name="wos", bufs=2) as wos,
            tc.tile_pool(name="osb", bufs=3) as osb,
            tc.tile_pool(name="sp", bufs=3, space="PSUM") as sp,
            tc.tile_pool(name="avp", bufs=2, space="PSUM") as avp,
            tc.tile_pool(name="dram", bufs=1, space="DRAM") as dram,
        ):
            # ---- warmup source (no DMA dependency) ----
            warm = cst.tile([128, SW], bf16)
            nc.vector.memset(warm[:], 0.25)
            ones_row = cst.tile([1, 128], bf16)
            nc.vector.memset(ones_row[:], 1.0)
            ones65 = cst.tile([65, DK], bf16)
            nc.vector.memset(ones65[:], 1.0)

            # preload the exp table set during the DMA phase
            bar_sb = cst.tile([1, 8], f32)
            nc.vector.memset(bar_sb[:], 0.0)
            dum = cst.tile([1, 8], f32)
            nc.scalar.activation(dum[:], bar_sb[:], EXP)

            # ---- start-of-kernel barrier input (absorbs launch skew); the
            # collective trigger is emitted after the input DMAs so it does
            # not block the gpsimd engine's DMA issue stream ----
            bar_in = dram.tile([1, 8], f32, tag="bar_in")
            bar_out = dram.tile([1, 8], f32, tag="bar_out")
            nc.sync.dma_start(bar_in[:], bar_sb[:])

            # ---- constants (scalar HWDGE ring) ----
            bq_sb = cst.tile([DPC, 1], f32)
            bk_sb = cst.tile([DPC, 1], f32)
            bv1_sb = cst.tile([1, VW], bf16)
            bo_sb = cst.tile([128, 8, 1], f32)
            nc.scalar.dma_start(bk_sb[:], bkd[:])
            nc.scalar.dma_start(bq_sb[:], bqd[:])
            nc.scalar.dma_start(bv1_sb[:], bv1[:])
            nc.scalar.dma_start(bo_sb[:], bod[:, :, :])
            mpat_sb = cst.tile([KT, n_pat, SW], bf16)
            nc.scalar.dma_start(mpat_sb[:], mpat[:, :, :])
            # wk/wv per-dt chunks interleaved: the first projection pair
            # needs (wk[dt], wv[dt]) in lockstep with the y quarters
            wq_sb = cst.tile([128, 8, DPC], bf16)
            wk_sb = cst.tile([128, 8, DPC], bf16)
            wv_sb = cst.tile([128, 8, VW], bf16)
            wo_sb = cst.tile([128, 8, DM], bf16)
            for dt in range(8):
                nc.scalar.dma_start(wk_sb[:, dt:dt + 1, :], wk[:, dt:dt + 1, :])
                nc.scalar.dma_start(wv_sb[:, dt:dt + 1, :], wv[:, dt:dt + 1, :])
            nc.scalar.dma_start(wq_sb[:], wq[:, :, :])

            # ---- p-state warmup: dependency-free matmuls keep the PE busy
            # (and ramping to full clock) while the input DMAs land ----
            wps = [sp.tile([128, 1024], f32, tag="sp", name=f"warm{i}")
                   for i in range(2)]
            for i in range(N_WARM):
                nc.tensor.matmul(wps[i % 2][:, :SW], warm[:, :128],
                                 warm[:, :SW], start=True, stop=True)

            qT = [big.tile([DPC, L], bf16, tag=f"qT{n}", name=f"qT{n}") for n in range(NB)]
            kT = [big.tile([DPC, L], bf16, tag=f"kT{n}", name=f"kT{n}") for n in range(NB)]
            vaug = [big.tile([128, NKT * VW], bf16, tag=f"va{n}", name=f"va{n}") for n in range(NB)]
            headT = [[big.tile([DK, L], bf16, tag=f"hT{n}{hp}", name=f"hT{n}{hp}")
                      for hp in range(HPC)] for n in range(NB)]

            a2a_in = [dram.tile([NC, DPC, CB], bf16, tag=f"a2ai{n}", name=f"a2ai{n}")
                      for n in range(NB)]
            a2a_out = [dram.tile([NC, DPC, CB], bf16, tag=f"a2ao{n}", name=f"a2ao{n}")
                       for n in range(NB)]
            # batch-1 collective is split in two halves with an interleaved
            # column->core map (each dest core takes 64 cols from each of the
            # half's two sub-windows), so half 0 fires right after qs1
            a2a1_in = [dram.tile([NC, DPC, 128], bf16, tag=f"a2b{h}", name=f"a2b{h}")
                       for h in range(2)]
            a2a1_out = [dram.tile([NC, DPC, 128], bf16, tag=f"a2c{h}", name=f"a2c{h}")
                        for h in range(2)]

            # ---- input tiles ----
            # (n=0, ch=0): 128KB quarters [128, 512] per (src, b, dt) so the
            # first projection block starts as early as possible, y on the
            # sync queue / x on the vector queue.
            # Later (n, ch): 1MB tiles [128, 4, 1024] (2KB descriptor runs),
            # spread across sync (y), vector (x) and gpsimd (spill) queues.
            srcmap = {}   # (src_name, n, b, dt) -> lambda(c0, c1) -> AP

            def emit_inputs():
                for src, nm, eng in ((ytb, "y", nc.sync), (xtb, "x", nc.scalar)):
                    for b in range(2):
                        for dt in range(8):
                            t = xyq.tile([128, SW], bf16, tag="xyq",
                                         name=f"{nm}q{b}{dt}")
                            eng.dma_start(
                                t[:], src[dt * 128:(dt + 1) * 128,
                                          b * SW:(b + 1) * SW])
                            srcmap[(nm, 0, b, dt)] = (
                                lambda t=t: lambda c0, c1: t[:, c0:c1])()

                def big_tile(src, nm, engs, n, ch):
                    for hf in range(2):
                        t = xy.tile([128, 4, 2 * SW], bf16, tag="xy",
                                    name=f"{nm}{n}c{ch}h{hf}")
                        engs[hf].dma_start(
                            t[:], src[hf * 512:(hf + 1) * 512,
                                      n * L + ch * 1024:n * L + (ch + 1) * 1024]
                            .rearrange("(t p) c -> p t c", p=128))
                        for b in (2 * ch, 2 * ch + 1):
                            for dt in (range(4) if hf == 0 else range(4, 8)):
                                srcmap[(nm, n, b, dt)] = (
                                    lambda t=t, dt=dt, b=b:
                                    lambda c0, c1: t[:, dt % 4,
                                                     (b % 2) * SW + c0:
                                                     (b % 2) * SW + c1])()
                # y stream on the sync ring, x stream on the scalar ring,
                # each in earliest-need order; the most start-critical ch1
                # tile (y0c1h0) rides the otherwise-idle gpsimd SWDGE queue
                big_tile(ytb, "y", (nc.sync, nc.sync), 0, 1)
                big_tile(xtb, "x", (nc.scalar, nc.scalar), 0, 1)
                big_tile(ytb, "y", (nc.sync, nc.sync), 1, 0)
                big_tile(xtb, "x", (nc.scalar, nc.scalar), 1, 0)
                big_tile(ytb, "y", (nc.sync, nc.sync), 1, 1)
                big_tile(xtb, "x", (nc.scalar, nc.scalar), 1, 1)
                nc.scalar.dma_start(wo_sb[:], wo[:, :, :])

            def ysl(n, b, dt, c0, c1):
                return srcmap[("y", n, b, dt)](c0, c1)

            def xsl(n, b, dt, c0, c1):
                return srcmap[("x", n, b, dt)](c0, c1)

            # ---- deferred attention finishers ----
            # stage a (den copies, vector) is emitted at the START of the
            # next block so the in-order vector queue runs it immediately;
            # stage b (broadcast matmul + reciprocal + normalize) after the
            # next block's first matmul group, by which time den is ready
            pending = []

            def flush_pending_a():
                for p in pending:
                    if p[0] is not None:
                        p[0]()
                        p[0] = None

            def flush_pending():
                flush_pending_a()
                while pending:
                    pending.pop(0)[1]()

            def _v_chain(n, b, j, psv, dt, last):
                # one step of a V-projection chain (stationary = y k-tile)
                if dt < 8:
                    nc.tensor.matmul(psv[:, :VW], ysl(n, b, dt, j * KT, (j + 1) * KT),
                                     wv_sb[:, dt, :], start=(dt == 0), stop=False)
                else:
                    nc.tensor.matmul(psv[:, :VW], ones_row[:], bv1_sb[:],
                                     start=False, stop=True)

            def emit_proj_kv(n, b):
                """K + V chains only (y-dependent). Used for the first two
                blocks so their x-dependent Q chains can be deferred until
                the x quarters have landed."""
                flush_pending_a()
                ps_k = sp.tile([128, 1024], f32, tag="sp", name=f"kkp{n}{b}")
                psv0 = sp.tile([128, 1024], f32, tag="sp", name=f"kvp{n}{b}0")
                for dt in range(9):
                    if dt < 8:
                        nc.tensor.matmul(ps_k[:DPC, :SW], wk_sb[:, dt, :],
                                         ysl(n, b, dt, 0, SW),
                                         start=(dt == 0), stop=(dt == 7))
                    _v_chain(n, b, 0, psv0, dt, False)
                nc.vector.tensor_scalar_add(kT[n][:, b * SW:(b + 1) * SW],
                                            ps_k[:DPC, :SW], bk_sb[:])
                nc.vector.tensor_copy(vaug[n][:, (b * 4) * VW:(b * 4 + 1) * VW],
                                      psv0[:, :VW])
                psv1 = sp.tile([128, 1024], f32, tag="sp", name=f"kvp{n}{b}1")
                psv2 = sp.tile([128, 1024], f32, tag="sp", name=f"kvp{n}{b}2")
                for dt in range(9):
                    _v_chain(n, b, 1, psv1, dt, False)
                    _v_chain(n, b, 2, psv2, dt, False)
                nc.vector.tensor_copy(vaug[n][:, (b * 4 + 1) * VW:(b * 4 + 2) * VW],
                                      psv1[:, :VW])
                nc.vector.tensor_copy(vaug[n][:, (b * 4 + 2) * VW:(b * 4 + 3) * VW],
                                      psv2[:, :VW])

            def emit_proj_q(n, b):
                """Deferred V3 + Q pair for a block emitted via emit_proj_kv."""
                psv3 = sp.tile([128, 1024], f32, tag="sp", name=f"kvp{n}{b}3")
                ps_q = sp.tile([128, 1024], f32, tag="sp", name=f"kqp{n}{b}")
                for dt in range(9):
                    _v_chain(n, b, 3, psv3, dt, False)
                    if dt < 8:
                        nc.tensor.matmul(ps_q[:DPC, :SW], wq_sb[:, dt, :],
                                         xsl(n, b, dt, 0, SW),
                                         start=(dt == 0), stop=(dt == 7))
                nc.vector.tensor_copy(vaug[n][:, (b * 4 + 3) * VW:(b * 4 + 4) * VW],
                                      psv3[:, :VW])
                nc.vector.tensor_scalar_add(qT[n][:, b * SW:(b + 1) * SW],
                                            ps_q[:DPC, :SW], bq_sb[:])

            def emit_proj_block(n, b):
                # chains interleaved in PAIRS so consecutive matmuls hit
                # alternating PSUM banks (avoids same-bank drain stalls)
                flush_pending_a()
                # pair 1: K chain & V chain j=0
                ps_k = sp.tile([128, 1024], f32, tag="sp", name=f"kp{n}{b}")
                psv0 = sp.tile([128, 1024], f32, tag="sp", name=f"vp{n}{b}0")
                for dt in range(9):
                    if dt < 8:
                        nc.tensor.matmul(ps_k[:DPC, :SW], wk_sb[:, dt, :],
                                         ysl(n, b, dt, 0, SW),
                                         start=(dt == 0), stop=(dt == 7))
                    _v_chain(n, b, 0, psv0, dt, False)
                nc.vector.tensor_scalar_add(kT[n][:, b * SW:(b + 1) * SW],
                                            ps_k[:DPC, :SW], bk_sb[:])
                nc.vector.tensor_copy(vaug[n][:, (b * 4) * VW:(b * 4 + 1) * VW],
                                      psv0[:, :VW])
                flush_pending()
                # pair 2: V chains j=1 & j=2
                psv1 = sp.tile([128, 1024], f32, tag="sp", name=f"vp{n}{b}1")
                psv2 = sp.tile([128, 1024], f32, tag="sp", name=f"vp{n}{b}2")
                for dt in range(9):
                    _v_chain(n, b, 1, psv1, dt, False)
                    _v_chain(n, b, 2, psv2, dt, False)
                nc.vector.tensor_copy(vaug[n][:, (b * 4 + 1) * VW:(b * 4 + 2) * VW],
                                      psv1[:, :VW])
                nc.vector.tensor_copy(vaug[n][:, (b * 4 + 2) * VW:(b * 4 + 3) * VW],
                                      psv2[:, :VW])
                # pair 3: V chain j=3 & Q chain
                psv3 = sp.tile([128, 1024], f32, tag="sp", name=f"vp{n}{b}3")
                ps_q = sp.tile([128, 1024], f32, tag="sp", name=f"qp{n}{b}")
                for dt in range(9):
                    _v_chain(n, b, 3, psv3, dt, False)
                    if dt < 8:
                        nc.tensor.matmul(ps_q[:DPC, :SW], wq_sb[:, dt, :],
                                         xsl(n, b, dt, 0, SW),
                                         start=(dt == 0), stop=(dt == 7))
                nc.vector.tensor_copy(vaug[n][:, (b * 4 + 3) * VW:(b * 4 + 4) * VW],
                                      psv3[:, :VW])
                nc.vector.tensor_scalar_add(qT[n][:, b * SW:(b + 1) * SW],
                                            ps_q[:DPC, :SW], bq_sb[:])

            def emit_attn_qs(n, qs):
                """Scores + exp + AV chains; queues the normalization as a
                deferred finisher (flushed inside the next block's stream)."""
                kts = [kt for kt in range(NKT) if cls_[qs][kt]]
                exp_tiles = {}
                flush_pending_a()
                avs = [avp.tile([65, SW], f32, tag="avp", name=f"av{n}{qs}{hp}")
                       for hp in range(HPC)]

                def emit_scores(kt):
                    a, b = span_[qs][kt]
                    ps = sp.tile([128, 1024], f32, tag="sp", name=f"s{n}{qs}{kt}")
                    for hp in range(HPC):
                        hs = hp * DK
                        nc.tensor.matmul(
                            ps[:KT, hp * SW + a:hp * SW + b],
                            kT[n][hs:hs + DK, kt * KT:(kt + 1) * KT],
                            qT[n][hs:hs + DK, qs * SW + a:qs * SW + b],
                            start=True, stop=True)
                    et = expp.tile([128, 2, SW], bf16, tag="exp", name=f"e{n}{qs}{kt}")
                    nc.scalar.activation(
                        et[:, :, a:b],
                        ps.rearrange("p (h c) -> p h c", h=2)[:, :, a:b], EXP)
                    if cls_[qs][kt] == 2:
                        pi = pidx[qs][kt]
                        for hp in range(HPC):
                            nc.vector.tensor_tensor(
                                et[:, hp, a:b], et[:, hp, a:b],
                                mpat_sb[:, pi, a:b], MUL)
                    exp_tiles[kt] = et

                def emit_av(kt):
                    # AV chain step: the two head chains interleaved so
                    # consecutive matmuls alternate PSUM banks
                    i = kts.index(kt)
                    a, b = span_[qs][kt]
                    for hp in range(HPC):
                        nc.tensor.matmul(
                            avs[hp][:, a:b],
                            vaug[n][:, kt * VW + hp * 65:kt * VW + (hp + 1) * 65],
                            exp_tiles[kt][:, hp, a:b],
                            start=(i == 0), stop=(i == len(kts) - 1))

                # software pipeline in 2-kt groups: AV lags scores/exp by one
                # group, so the post-exp serial tail is just the last group
                groups = [kts[g:g + 2] for g in range(0, len(kts), 2)]
                prev = None
                first = True
                for group in groups:
                    for kt in group:
                        emit_scores(kt)
                    if first:
                        # after the first score group the previous attention
                        # block's AV psum tiles are no longer the PE's next
                        # dependency: finish (normalize) it now
                        flush_pending()
                        first = False
                    if prev is not None:
                        for kt in prev:
                            emit_av(kt)
                    prev = group
                for kt in prev:
                    emit_av(kt)

                dens = [nrm.tile([65, SW], bf16, tag="den", name=f"dn{n}{qs}{hp}")
                        for hp in range(HPC)]

                def finish_a():
                    for hp in range(HPC):
                        nc.vector.tensor_copy(dens[hp][64:65, :],
                                              avs[hp][64:65, :])

                def finish_b():
                    # both broadcast matmuls first, then per-head recip+norm
                    # (head 0 completes ~1us earlier, unblocking its a2a
                    # staging DMA)
                    bcs = []
                    for hp in range(HPC):
                        bc = sp.tile([DK, SW], f32, tag="sp", name=f"bc{n}{qs}{hp}")
                        nc.tensor.matmul(bc[:], ones65[64:65, :],
                                         dens[hp][64:65, :],
                                         start=True, stop=True)
                        bcs.append(bc)
                    for hp in range(HPC):
                        rec = nrm.tile([DK, SW], f32, tag="rec", name=f"rc{n}{qs}{hp}")
                        nc.vector.reciprocal_approx_fast(rec[:], bcs[hp][:])
                        nc.vector.tensor_tensor(
                            headT[n][hp][:, qs * SW:(qs + 1) * SW],
                            avs[hp][:DK, :], rec[:], MUL)
                pending.append([finish_a, finish_b])

            def emit_a2a(n):
                for hp in range(HPC):
                    nc.sync.dma_start(
                        a2a_in[n][:, hp * DK:(hp + 1) * DK, :].transpose([1, 0, 2]),
                        headT[n][hp][:, :].rearrange("p (j c) -> p j c", j=NC))
                nc.gpsimd.collective_compute(
                    "AllToAll", mybir.AluOpType.bypass,
                    replica_groups=[list(range(NC))],
                    ins=[a2a_in[n].opt()], outs=[a2a_out[n].opt()])

            def emit_a2a1_half(h):
                # cols of half h: local col = q*512 + j*64 + c  (q: sub-window
                # within half, j: dest core, c: 0..63)
                for hp in range(HPC):
                    nc.sync.dma_start(
                        a2a1_in[h][:, hp * DK:(hp + 1) * DK, :]
                        .rearrange("j p (q c) -> p q j c", q=2),
                        headT[1][hp][:, h * 1024:(h + 1) * 1024]
                        .rearrange("p (q j c) -> p q j c", q=2, j=NC))
                nc.gpsimd.collective_compute(
                    "AllToAll", mybir.AluOpType.bypass,
                    replica_groups=[list(range(NC))],
                    ins=[a2a1_in[h].opt()], outs=[a2a1_out[h].opt()])

            def fetch_rhs(n):
                # per-jj rhs loads so the first matmuls start as soon as the
                # first 32KB chunk lands (matters after the last collective)
                if n == 0:
                    src, ncols, tagn = a2a_out[0], CB, "r0"
                else:
                    src, ncols, tagn = a2a1_out[n - 1], 128, f"r1{n-1}"
                rhs_t = wos.tile([128, NC, CB], bf16, tag="rhs", name=f"rhs{tagn}")
                for jj in range(8):
                    nc.sync.dma_start(rhs_t[:, jj:jj + 1, :ncols],
                                      src[jj:jj + 1, :, :ncols].transpose([1, 0, 2]))
                return rhs_t

            def emit_wo(n, rhs_t=None):
                if n == 0:
                    ncols, col0, tagn = CB, 0, "r0"
                else:
                    h = n - 1
                    ncols, col0, tagn = 128, CB + h * 128, f"r1{h}"
                if rhs_t is None:
                    rhs_t = fetch_rhs(n)
                flush_pending_a()
                fl = False
                groups = ((0, 1, 2), (3, 4, 5), (6,), (7,)) if n == 2 \
                    else ((0, 1, 2), (3, 4, 5), (6, 7))
                for mts in groups:
                    pss = [sp.tile([128, 1024], f32, tag="sp", name=f"wp{tagn}{mt}")
                           for mt in mts]
                    for jj in range(8):
                        for k, mt in enumerate(mts):
                            nc.tensor.matmul(pss[k][:, :ncols],
                                             wo_sb[:, jj, mt * KT:(mt + 1) * KT],
                                             rhs_t[:, jj, :ncols],
                                             start=(jj == 0), stop=(jj == 7))
                    if not fl:
                        flush_pending()
                        fl = True
                    for k, mt in enumerate(mts):
                        ob = osb.tile([128, CB], f32, tag="osb", name=f"ob{tagn}{mt}")
                        nc.vector.tensor_scalar_add(ob[:, :ncols], pss[k][:, :ncols],
                                                    bo_sb[:, mt, :])
                        # scalar ring: exp is finished by the (tail-only) WO
                        # phase, and this keeps the sync ring free for the
                        # collective staging + rhs chunk reads
                        nc.scalar.dma_start(out_t[mt * KT:(mt + 1) * KT, col0:col0 + ncols],
                                            ob[:, :ncols])

            # ---- pipeline: attention interleaved between projection blocks;
            # WO chunks emitted as soon as their collective lands ----
            emit_inputs()
            nc.gpsimd.collective_compute(
                "AllReduce", mybir.AluOpType.add,
                replica_groups=[list(range(NC))],
                ins=[bar_in.opt()], outs=[bar_out.opt()])
            # projection blocks emitted in PAIRS (10-13us of dense matmuls,
            # so the second block runs at the fully-ramped clock) separated
            # by runs of attention blocks (exp-paced on ScalarE regardless
            # of PE clock)
            emit_proj_kv(0, 0)
            emit_proj_kv(0, 1)
            emit_proj_q(0, 0)
            emit_proj_q(0, 1)
            emit_attn_qs(0, 0)
            emit_proj_block(0, 2)      # flushes finish(0,0)
            emit_proj_block(0, 3)
            emit_attn_qs(0, 1)
            emit_attn_qs(0, 2)         # flushes finish(0,1)
            emit_proj_block(1, 0)      # flushes finish(0,2)
            emit_proj_block(1, 1)
            emit_attn_qs(0, 3)
            emit_attn_qs(1, 0)         # flushes finish(0,3)
            emit_a2a(0)
            emit_proj_block(1, 2)      # flushes finish(1,0)
            emit_proj_block(1, 3)
            emit_attn_qs(1, 1)
            flush_pending()            # finish(1,1) before its collective
            emit_a2a1_half(0)
            emit_attn_qs(1, 2)
            rhs0 = fetch_rhs(0)        # prefetch: sync ring is idle here and
                                       # a2a(0) completed long ago, so wo(0)
                                       # can start the instant it's reached
            emit_attn_qs(1, 3)         # flushes finish(1,2) internally
            # finish(1,3): den copies first, a few dependency-free matmuls
            # bridge the copy latency so the PE clock stays hot, then the
            # broadcast+normalize chain and the final collective's staging
            flush_pending_a()
            hot = [sp.tile([128, 1024], f32, tag="sp", name=f"hot{i}")
                   for i in range(2)]
            for i in range(7):
                nc.tensor.matmul(hot[i % 2][:, :SW], warm[:, :128],
                                 warm[:, :SW], start=True, stop=True)
            flush_pending()
            emit_a2a1_half(1)
            # wo(0) and wo(1) are deferred to AFTER the final collective's
            # staging: ~20us of collective-independent PE work that absorbs
            # peer launch skew while a2a1h1 completes
            emit_wo(0, rhs0)
            rhs1 = fetch_rhs(1)
            emit_wo(1, rhs1)
            emit_wo(2)

    nc.compile()
    return nc


def kernel(x, y, mask, Wq, bq, Wk, bk, Wv, bv, Wo, bo, _trace=False):
    x = np.asarray(x, np.float32)
    y = np.asarray(y, np.float32)
    cls_, span_, pidx, pats = _classify_blocks(mask)

    key = (x.shape,
           tuple(tuple(c) for c in cls_),
           tuple(tuple(s) for s in span_),
           tuple(tuple(p) for p in pidx),
           pats.tobytes())
    if key not in _CACHE:
        _CACHE[key] = _build(cls_, span_, pidx, pats.shape[0])
    nc = _CACHE[key]

    fac = np.float32(1.0 / np.sqrt(DK))
    xtb = np.ascontiguousarray(
        np.concatenate([x[n].T for n in range(NB)], axis=1)).astype(BF16)
    ytb = np.ascontiguousarray(
        np.concatenate([y[n].T for n in range(NB)], axis=1)).astype(BF16)
    Wq32 = np.asarray(Wq, np.float32) * fac
    bq32 = np.asarray(bq, np.float32) * fac

    def pmajor(w):
        # [DM, X] -> [128, 8, X] with [p, t, :] = w[t*128+p, :]
        w = np.asarray(w)
        return np.ascontiguousarray(w.reshape(8, 128, w.shape[1]).transpose(1, 0, 2))

    wo_pm = pmajor(np.asarray(Wo, np.float32)).astype(BF16)
    bo_pm = pmajor(np.asarray(bo, np.float32).reshape(DM, 1))
    mpat_t = np.ascontiguousarray(pats.transpose(1, 0, 2))

    in_maps = []
    for c in range(NC):
        d0 = c * DPC
        wv_aug = np.zeros((DM, VW), np.float32)
        bv1 = np.zeros((1, VW), np.float32)
        for hp in range(HPC):
            h = HPC * c + hp
            wv_aug[:, hp * 65:hp * 65 + DK] = np.asarray(Wv, np.float32)[:, h * DK:(h + 1) * DK]
            bv1[0, hp * 65:hp * 65 + DK] = np.asarray(bv, np.float32)[h * DK:(h + 1) * DK]
            bv1[0, hp * 65 + DK] = 1.0
        in_maps.append({
            "xtb": xtb, "ytb": ytb,
            "wq": pmajor(Wq32[:, d0:d0 + DPC]).astype(BF16),
            "wk": pmajor(np.asarray(Wk, np.float32)[:, d0:d0 + DPC]).astype(BF16),
            "wv": pmajor(wv_aug).astype(BF16),
            "wo": wo_pm,
            "bq": bq32[d0:d0 + DPC].reshape(DPC, 1),
            "bk": np.asarray(bk, np.float32)[d0:d0 + DPC].reshape(DPC, 1),
            "bv1": bv1.astype(BF16),
            "bo": bo_pm,
            "mpat": mpat_t,
        })

    res = run_bass_kernel_spmd(nc, in_maps, core_ids=list(range(NC)), trace=_trace)
    out = np.empty((NB, L, DM), np.float32)
    for c in range(NC):
        ot = res.results[c]["out_t"]
        out[0, c * CB:(c + 1) * CB, :] = ot[:, :CB].T
        # batch 1: interleaved col->core map, 64 rows per (half, sub-window)
        for h in range(2):
            for q in range(2):
                r0 = (2 * h + q) * SW + c * 64
                out[1, r0:r0 + 64, :] = ot[:, CB + h * 128 + q * 64:
                                           CB + h * 128 + (q + 1) * 64].T
    if _trace:
        kernel.last_results = res
    return out


# revision 42
# speedup vs baseline: 1.1043x; 1.1043x over previous
"""Multi-head attention (N=2, L=2048, 16 heads x 64) on 8 TRN2 NeuronCores.

Head-parallel attention (2 heads/core) with a per-batch software pipeline:
attention emission interleaved between projection blocks, per-batch AllToAll
(head-split -> sequence-split) hidden under compute, output projection
spread across the stream.

Scheduling (v2):
- PE p-state warmup: ~44 dependency-free matmuls issued at kernel start so
  the tensor engine ramps to full clock during the unavoidable launch+DMA
  window; real matmuls then start hot.
- Inputs split across four HWDGE queues (sync=y, vector=x, scalar=weights,
  gpsimd=spill+outputs); the first projection block's tiles arrive as
  128KB quarters so the first real matmul fires as soon as possible.
- Deferred softmax finish: the den-copy -> broadcast-matmul -> reciprocal
  -> normalize chain of attention block N is emitted inside block N+1's
  matmul stream, removing the per-block PE stall (and its p-state reset).
- Output projection chunks emitted mid-stream right after their collective
  lands; only the last 128-col chunk trails the final collective.
- Softmax normalization without DMA round trips: ones-column in V gives the
  denominator as AV row 64; K=1 matmul broadcasts it,
  reciprocal_approx_fast + one DVE multiply normalize.
- exp is one ScalarE call per k-tile covering both heads via a 3D AP.
"""
import sys

sys.path.insert(0, "/opt/trn_rl_repo")

import numpy as np
import ml_dtypes

import concourse.bass as bass
import concourse.bacc as bacc
import concourse.mybir as mybir
import concourse.tile as tile
from concourse.bass_utils import run_bass_kernel_spmd

BF16 = ml_dtypes.bfloat16

DM = 1024      # dmodel
DK = 64        # head dim
H = 16         # heads
NB = 2         # batch
L = 2048       # seq len
R = NB * L
NC = 8         # cores
HPC = H // NC  # heads per core = 2
DPC = HPC * DK  # depth per core = 128

SW = 512       # q sub-window
KT = 128       # k tile
NQS = L // SW   # 4 q blocks per batch
NKT = L // KT   # 16 k tiles per batch
CB = L // NC    # 256: per-batch per-core output chunk
VW = 65 * HPC   # 130: augmented v width (both heads, +ones col each)

N_WARM = 48    # p-state warmup matmuls (ap=512 each)

_CACHE = {}


def _classify_blocks(mask):
    """Per (qs, kt) block: 0=skip, 1=full, 2=partial (+ q-span, pattern)."""
    mask = np.asarray(mask, dtype=bool)
    cls = [[0] * NKT for _ in range(NQS)]
    span = [[None] * NKT for _ in range(NQS)]
    pat_ids = {}
    pats = []
    pat_idx = [[-1] * NKT for _ in range(NQS)]
    for qs in range(NQS):
        for kt in range(NKT):
            sub = mask[qs * SW:(qs + 1) * SW, kt * KT:(kt + 1) * KT]
            rows = np.nonzero(sub.any(axis=1))[0]
            if rows.size == 0:
                cls[qs][kt] = 0
            elif sub.all():
                cls[qs][kt] = 1
                span[qs][kt] = (0, SW)
            else:
                cls[qs][kt] = 2
                span[qs][kt] = (int(rows[0]), int(rows[-1]) + 1)
                pat = np.ascontiguousarray(sub.T).astype(BF16)  # [128 k, SW q]
                key = pat.tobytes()
                if key not in pat_ids:
                    pat_ids[key] = len(pats)
                    pats.append(pat)
                pat_idx[qs][kt] = pat_ids[key]
    # the first included kt of each sub-window must cover the full 512
    # columns (its start=True matmul clears PSUM has_written)
    for qs in range(NQS):
        for kt in range(NKT):
            if cls[qs][kt]:
                span[qs][kt] = (0, SW)
                break
    if not pats:
        pats.append(np.ones((KT, SW), dtype=BF16))
    return cls, span, pat_idx, np.stack(pats)


def _build(cls_, span_, pidx, n_pat):
    nc = bacc.Bacc("TRN2", target_bir_lowering=False, debug=False,
                   enable_asserts=False, num_devices=NC)
    f32, bf16 = mybir.dt.float32, mybir.dt.bfloat16
    EXP = mybir.ActivationFunctionType.Exp
    MUL = mybir.AluOpType.mult

    # weights arrive host-pre-shuffled into partition-major layouts so every
    # const DMA is one contiguous chunk per partition (few, large descriptors)
    xtb = nc.dram_tensor("xtb", [DM, R], bf16, kind="ExternalInput")
    ytb = nc.dram_tensor("ytb", [DM, R], bf16, kind="ExternalInput")
    wq = nc.dram_tensor("wq", [128, 8, DPC], bf16, kind="ExternalInput")
    wk = nc.dram_tensor("wk", [128, 8, DPC], bf16, kind="ExternalInput")
    wv = nc.dram_tensor("wv", [128, 8, VW], bf16, kind="ExternalInput")
    wo = nc.dram_tensor("wo", [128, 8, DM], bf16, kind="ExternalInput")
    bqd = nc.dram_tensor("bq", [DPC, 1], f32, kind="ExternalInput")
    bkd = nc.dram_tensor("bk", [DPC, 1], f32, kind="ExternalInput")
    bv1 = nc.dram_tensor("bv1", [1, VW], bf16, kind="ExternalInput")
    bod = nc.dram_tensor("bo", [128, 8, 1], f32, kind="ExternalInput")
    mpat = nc.dram_tensor("mpat", [KT, n_pat, SW], bf16, kind="ExternalInput")
    out_t = nc.dram_tensor("out_t", [DM, NB * CB], f32, kind="ExternalOutput")

    with tile.TileContext(nc) as tc:
        with (
            tc.tile_pool(name="cst", bufs=1) as cst,
            tc.tile_pool(name="xyq", bufs=32) as xyq,
            tc.tile_pool(name="xy", bufs=6) as xy,
            tc.tile_pool(name="big", bufs=1) as big,
            tc.tile_pool(name="expp", bufs=12) as expp,
            tc.tile_pool(name="nrm", bufs=3) as nrm,
            tc.tile_pool(name="wos", bufs=2) as wos,
            tc.tile_pool(name="osb", bufs=3) as osb,
            tc.tile_pool(name="sp", bufs=3, space="PSUM") as sp,
            tc.tile_pool(name="avp", bufs=2, space="PSUM") as avp,
            tc.tile_pool(name="dram", bufs=1, space="DRAM") as dram,
        ):
            # ---- warmup source (no DMA dependency) ----
            warm = cst.tile([128, SW], bf16)
            nc.vector.memset(warm[:], 0.25)
            ones_row = cst.tile([1, 128], bf16)
            nc.vector.memset(ones_row[:], 1.0)
            ones65 = cst.tile([65, DK], bf16)
            nc.vector.memset(ones65[:], 1.0)

            # preload the exp table set during the DMA phase
            bar_sb = cst.tile([1, 8], f32)
            nc.vector.memset(bar_sb[:], 0.0)
            dum = cst.tile([1, 8], f32)
            nc.scalar.activation(dum[:], bar_sb[:], EXP)

            # ---- start-of-kernel barrier input (absorbs launch skew); the
            # collective trigger is emitted after the input DMAs so it does
            # not block the gpsimd engine's DMA issue stream ----
            bar_in = dram.tile([1, 8], f32, tag="bar_in")
            bar_out = dram.tile([1, 8], f32, tag="bar_out")
            nc.sync.dma_start(bar_in[:], bar_sb[:])

            # ---- constants (scalar HWDGE ring) ----
            bq_sb = cst.tile([DPC, 1], f32)
            bk_sb = cst.tile([DPC, 1], f32)
            bv1_sb = cst.tile([1, VW], bf16)
            bo_sb = cst.tile([128, 8, 1], f32)
            nc.scalar.dma_start(bk_sb[:], bkd[:])
            nc.scalar.dma_start(bq_sb[:], bqd[:])
            nc.scalar.dma_start(bv1_sb[:], bv1[:])
            nc.scalar.dma_start(bo_sb[:], bod[:, :, :])
            mpat_sb = cst.tile([KT, n_pat, SW], bf16)
            nc.scalar.dma_start(mpat_sb[:], mpat[:, :, :])
            # wk/wv per-dt chunks interleaved: the first projection pair
            # needs (wk[dt], wv[dt]) in lockstep with the y quarters
            wq_sb = cst.tile([128, 8, DPC], bf16)
            wk_sb = cst.tile([128, 8, DPC], bf16)
            wv_sb = cst.tile([128, 8, VW], bf16)
            wo_sb = cst.tile([128, 8, DM], bf16)
            for dt in range(8):
                nc.scalar.dma_start(wk_sb[:, dt:dt + 1, :], wk[:, dt:dt + 1, :])
                nc.scalar.dma_start(wv_sb[:, dt:dt + 1, :], wv[:, dt:dt + 1, :])
            nc.scalar.dma_start(wq_sb[:], wq[:, :, :])

            # ---- p-state warmup: dependency-free matmuls keep the PE busy
            # (and ramping to full clock) while the input DMAs land ----
            wps = [sp.tile([128, 1024], f32, tag="sp", name=f"warm{i}")
                   for i in range(2)]
            for i in range(N_WARM):
                nc.tensor.matmul(wps[i % 2][:, :SW], warm[:, :128],
                                 warm[:, :SW], start=True, stop=True)

            qT = [big.tile([DPC, L], bf16, tag=f"qT{n}", name=f"qT{n}") for n in range(NB)]
            kT = [big.tile([DPC, L], bf16, tag=f"kT{n}", name=f"kT{n}") for n in range(NB)]
            vaug = [big.tile([128, NKT * VW], bf16, tag=f"va{n}", name=f"va{n}") for n in range(NB)]
            headT = [[big.tile([DK, L], bf16, tag=f"hT{n}{hp}", name=f"hT{n}{hp}")
                      for hp in range(HPC)] for n in range(NB)]

            a2a_in = [dram.tile([NC, DPC, CB], bf16, tag=f"a2ai{n}", name=f"a2ai{n}")
                      for n in range(NB)]
            a2a_out = [dram.tile([NC, DPC, CB], bf16, tag=f"a2ao{n}", name=f"a2ao{n}")
                       for n in range(NB)]
            # batch-1 collective is split in two halves with an interleaved
            # column->core map (each dest core takes 64 cols from each of the
            # half's two sub-windows), so half 0 fires right after qs1
            a2a1_in = [dram.tile([NC, DPC, 128], bf16, tag=f"a2b{h}", name=f"a2b{h}")
                       for h in range(2)]
            a2a1_out = [dram.tile([NC, DPC, 128], bf16, tag=f"a2c{h}", name=f"a2c{h}")
                        for h in range(2)]

            # ---- input tiles ----
            # (n=0, ch=0): 128KB quarters [128, 512] per (src, b, dt) so the
            # first projection block starts as early as possible, y on the
            # sync queue / x on the vector queue.
            # Later (n, ch): 1MB tiles [128, 4, 1024] (2KB descriptor runs),
            # spread across sync (y), vector (x) and gpsimd (spill) queues.
            srcmap = {}   # (src_name, n, b, dt) -> lambda(c0, c1) -> AP

            def emit_inputs():
                for src, nm, eng in ((ytb, "y", nc.sync), (xtb, "x", nc.scalar)):
                    for b in range(2):
                        for dt in range(8):
                            t = xyq.tile([128, SW], bf16, tag="xyq",
                                         name=f"{nm}q{b}{dt}")
                            eng.dma_start(
                                t[:], src[dt * 128:(dt + 1) * 128,
                                          b * SW:(b + 1) * SW])
                            srcmap[(nm, 0, b, dt)] = (
                                lambda t=t: lambda c0, c1: t[:, c0:c1])()

                def big_tile(src, nm, engs, n, ch):
                    for hf in range(2):
                        t = xy.tile([128, 4, 2 * SW], bf16, tag="xy",
                                    name=f"{nm}{n}c{ch}h{hf}")
                        engs[hf].dma_start(
                            t[:], src[hf * 512:(hf + 1) * 512,
                                      n * L + ch * 1024:n * L + (ch + 1) * 1024]
                            .rearrange("(t p) c -> p t c", p=128))
                        for b in (2 * ch, 2 * ch + 1):
                            for dt in (range(4) if hf == 0 else range(4, 8)):
                                srcmap[(nm, n, b, dt)] = (
                                    lambda t=t, dt=dt, b=b:
                                    lambda c0, c1: t[:, dt % 4,
                                                     (b % 2) * SW + c0:
                                                     (b % 2) * SW + c1])()
                # y stream on the sync ring, x stream on the scalar ring,
                # each in earliest-need order; the most start-critical ch1
                # tile (y0c1h0) rides the otherwise-idle gpsimd SWDGE queue
                big_tile(ytb, "y", (nc.sync, nc.sync), 0, 1)
                big_tile(xtb, "x", (nc.scalar, nc.scalar), 0, 1)
                big_tile(ytb, "y", (nc.sync, nc.sync), 1, 0)
                big_tile(xtb, "x", (nc.scalar, nc.scalar), 1, 0)
                big_tile(ytb, "y", (nc.sync, nc.sync), 1, 1)
                big_tile(xtb, "x", (nc.scalar, nc.scalar), 1, 1)
                nc.scalar.dma_start(wo_sb[:], wo[:, :, :])

            def ysl(n, b, dt, c0, c1):
                return srcmap[("y", n, b, dt)](c0, c1)

            def xsl(n, b, dt, c0, c1):
                return srcmap[("x", n, b, dt)](c0, c1)

            # ---- deferred attention finishers ----
            # stage a (den copies, vector) is emitted at the START of the
            # next block so the in-order vector queue runs it immediately;
            # stage b (broadcast matmul + reciprocal + normalize) after the
            # next block's first matmul group, by which time den is ready
            pending = []

            def flush_pending_a():
                for p in pending:
                    if p[0] is not None:
                        p[0]()
                        p[0] = None

            def flush_pending():
                flush_pending_a()
                while pending:
                    pending.pop(0)[1]()

            def _v_chain(n, b, j, psv, dt, last):
                # one step of a V-projection chain (stationary = y k-tile)
                if dt < 8:
                    nc.tensor.matmul(psv[:, :VW], ysl(n, b, dt, j * KT, (j + 1) * KT),
                                     wv_sb[:, dt, :], start=(dt == 0), stop=False)
                else:
                    nc.tensor.matmul(psv[:, :VW], ones_row[:], bv1_sb[:],
                                     start=False, stop=True)

            def emit_proj_kv(n, b):
                """K + V chains only (y-dependent). Used for the first two
                blocks so their x-dependent Q chains can be deferred until
                the x quarters have landed."""
                flush_pending_a()
                ps_k = sp.tile([128, 1024], f32, tag="sp", name=f"kkp{n}{b}")
                psv0 = sp.tile([128, 1024], f32, tag="sp", name=f"kvp{n}{b}0")
                for dt in range(9):
                    if dt < 8:
                        nc.tensor.matmul(ps_k[:DPC, :SW], wk_sb[:, dt, :],
                                         ysl(n, b, dt, 0, SW),
                                         start=(dt == 0), stop=(dt == 7))
                    _v_chain(n, b, 0, psv0, dt, False)
                nc.vector.tensor_scalar_add(kT[n][:, b * SW:(b + 1) * SW],
                                            ps_k[:DPC, :SW], bk_sb[:])
                nc.vector.tensor_copy(vaug[n][:, (b * 4) * VW:(b * 4 + 1) * VW],
                                      psv0[:, :VW])
                psv1 = sp.tile([128, 1024], f32, tag="sp", name=f"kvp{n}{b}1")
                psv2 = sp.tile([128, 1024], f32, tag="sp", name=f"kvp{n}{b}2")
                for dt in range(9):
                    _v_chain(n, b, 1, psv1, dt, False)
                    _v_chain(n, b, 2, psv2, dt, False)
                nc.vector.tensor_copy(vaug[n][:, (b * 4 + 1) * VW:(b * 4 + 2) * VW],
                                      psv1[:, :VW])
                nc.vector.tensor_copy(vaug[n][:, (b * 4 + 2) * VW:(b * 4 + 3) * VW],
                                      psv2[:, :VW])

            def emit_proj_q(n, b):
                """Deferred V3 + Q pair for a block emitted via emit_proj_kv."""
                psv3 = sp.tile([128, 1024], f32, tag="sp", name=f"kvp{n}{b}3")
                ps_q = sp.tile([128, 1024], f32, tag="sp", name=f"kqp{n}{b}")
                for dt in range(9):
                    _v_chain(n, b, 3, psv3, dt, False)
                    if dt < 8:
                        nc.tensor.matmul(ps_q[:DPC, :SW], wq_sb[:, dt, :],
                                         xsl(n, b, dt, 0, SW),
                                         start=(dt == 0), stop=(dt == 7))
                nc.vector.tensor_copy(vaug[n][:, (b * 4 + 3) * VW:(b * 4 + 4) * VW],
                                      psv3[:, :VW])
                nc.vector.tensor_scalar_add(qT[n][:, b * SW:(b + 1) * SW],
                                            ps_q[:DPC, :SW], bq_sb[:])

            def emit_proj_block(n, b):
                # chains interleaved in PAIRS so consecutive matmuls hit
                # alternating PSUM banks (avoids same-bank drain stalls)
                flush_pending_a()
                # pair 1: K chain & V chain j=0
                ps_k = sp.tile([128, 1024], f32, tag="sp", name=f"kp{n}{b}")
                psv0 = sp.tile([128, 1024], f32, tag="sp", name=f"vp{n}{b}0")
                for dt in range(9):
                    if dt < 8:
                        nc.tensor.matmul(ps_k[:DPC, :SW], wk_sb[:, dt, :],
                                         ysl(n, b, dt, 0, SW),
                                         start=(dt == 0), stop=(dt == 7))
                    _v_chain(n, b, 0, psv0, dt, False)
                nc.vector.tensor_scalar_add(kT[n][:, b * SW:(b + 1) * SW],
                                            ps_k[:DPC, :SW], bk_sb[:])
                nc.vector.tensor_copy(vaug[n][:, (b * 4) * VW:(b * 4 + 1) * VW],
                                      psv0[:, :VW])
                flush_pending()
                # pair 2: V chains j=1 & j=2
                psv1 = sp.tile([128, 1024], f32, tag="sp", name=f"vp{n}{b}1")
                psv2 = sp.tile([128, 1024], f32, tag="sp", name=f"vp{n}{b}2")
                for dt in range(9):
                    _v_chain(n, b, 1, psv1, dt, False)
                    _v_chain(n, b, 2, psv2, dt, False)
                nc.vector.tensor_copy(vaug[n][:, (b * 4 + 1) * VW:(b * 4 + 2) * VW],
                                      psv1[:, :VW])
                nc.vector.tensor_copy(vaug[n][:, (b * 4 + 2) * VW:(b * 4 + 3) * VW],
                                      psv2[:, :VW])
                # pair 3: V chain j=3 & Q chain
                psv3 = sp.tile([128, 1024], f32, tag="sp", name=f"vp{n}{b}3")
                ps_q = sp.tile([128, 1024], f32, tag="sp", name=f"qp{n}{b}")
                for dt in range(9):
                    _v_chain(n, b, 3, psv3, dt, False)
                    if dt < 8:
                        nc.tensor.matmul(ps_q[:DPC, :SW], wq_sb[:, dt, :],
                                         xsl(n, b, dt, 0, SW),
                                         start=(dt == 0), stop=(dt == 7))
                nc.vector.tensor_copy(vaug[n][:, (b * 4 + 3) * VW:(b * 4 + 4) * VW],
                                      psv3[:, :VW])
                nc.vector.tensor_scalar_add(qT[n][:, b * SW:(b + 1) * SW],
                                            ps_q[:DPC, :SW], bq_sb[:])

            def emit_attn_qs(n, qs):
                """Scores + exp + AV chains; queues the normalization as a
                deferred finisher (flushed inside the next block's stream)."""
                kts = [kt for kt in range(NKT) if cls_[qs][kt]]
                exp_tiles = {}
                flush_pending_a()
                avs = [avp.tile([65, SW], f32, tag="avp", name=f"av{n}{qs}{hp}")
                       for hp in range(HPC)]

                def emit_scores(kt):
                    a, b = span_[qs][kt]
                    ps = sp.tile([128, 1024], f32, tag="sp", name=f"s{n}{qs}{kt}")
                    for hp in range(HPC):
                        hs = hp * DK
                        nc.tensor.matmul(
                            ps[:KT, hp * SW + a:hp * SW + b],
                            kT[n][hs:hs + DK, kt * KT:(kt + 1) * KT],
                            qT[n][hs:hs + DK, qs * SW + a:qs * SW + b],
                            start=True, stop=True)
                    et = expp.tile([128, 2, SW], bf16, tag="exp", name=f"e{n}{qs}{kt}")
                    nc.scalar.activation(
                        et[:, :, a:b],
                        ps.rearrange("p (h c) -> p h c", h=2)[:, :, a:b], EXP)
                    if cls_[qs][kt] == 2:
                        pi = pidx[qs][kt]
                        for hp in range(HPC):
                            nc.vector.tensor_tensor(
                                et[:, hp, a:b], et[:, hp, a:b],
                                mpat_sb[:, pi, a:b], MUL)
                    exp_tiles[kt] = et

                def emit_av(kt):
                    # AV chain step: the two head chains interleaved so
                    # consecutive matmuls alternate PSUM banks
                    i = kts.index(kt)
                    a, b = span_[qs][kt]
                    for hp in range(HPC):
                        nc.tensor.matmul(
                            avs[hp][:, a:b],
                            vaug[n][:, kt * VW + hp * 65:kt * VW + (hp + 1) * 65],
                            exp_tiles[kt][:, hp, a:b],
                            start=(i == 0), stop=(i == len(kts) - 1))

                # software pipeline in 2-kt groups: AV lags scores/exp by one
                # group, so the post-exp serial tail is just the last group
                groups = [kts[g:g + 2] for g in range(0, len(kts), 2)]
                prev = None
                first = True
                for group in groups:
                    for kt in group:
                        emit_scores(kt)
                    if first:
                        # after the first score group the previous attention
                        # block's AV psum tiles are no longer the PE's next
                        # dependency: finish (normalize) it now
                        flush_pending()
                        first = False
                    if prev is not None:
                        for kt in prev:
                            emit_av(kt)
                    prev = group
                for kt in prev:
                    emit_av(kt)

                dens = [nrm.tile([65, SW], bf16, tag="den", name=f"dn{n}{qs}{hp}")
                        for hp in range(HPC)]

                def finish_a():
                    for hp in range(HPC):
                        nc.vector.tensor_copy(dens[hp][64:65, :],
                                              avs[hp][64:65, :])

                def finish_b():
                    # both broadcast matmuls first, then per-head recip+norm
                    # (head 0 completes ~1us earlier, unblocking its a2a
                    # staging DMA)
                    bcs = []
                    for hp in range(HPC):
                        bc = sp.tile([DK, SW], f32, tag="sp", name=f"bc{n}{qs}{hp}")
                        nc.tensor.matmul(bc[:], ones65[64:65, :],
                                         dens[hp][64:65, :],
                                         start=True, stop=True)
                        bcs.append(bc)
                    for hp in range(HPC):
                        rec = nrm.tile([DK, SW], f32, tag="rec", name=f"rc{n}{qs}{hp}")
                        nc.vector.reciprocal_approx_fast(rec[:], bcs[hp][:])
                        nc.vector.tensor_tensor(
                            headT[n][hp][:, qs * SW:(qs + 1) * SW],
                            avs[hp][:DK, :], rec[:], MUL)
                pending.append([finish_a, finish_b])

            def emit_a2a(n):
                for hp in range(HPC):
                    nc.sync.dma_start(
                        a2a_in[n][:, hp * DK:(hp + 1) * DK, :].transpose([1, 0, 2]),
                        headT[n][hp][:, :].rearrange("p (j c) -> p j c", j=NC))
                nc.gpsimd.collective_compute(
                    "AllToAll", mybir.AluOpType.bypass,
                    replica_groups=[list(range(NC))],
                    ins=[a2a_in[n].opt()], outs=[a2a_out[n].opt()])

            def emit_a2a1_half(h):
                # cols of half h: local col = q*512 + j*64 + c  (q: sub-window
                # within half, j: dest core, c: 0..63)
                for hp in range(HPC):
                    nc.sync.dma_start(
                        a2a1_in[h][:, hp * DK:(hp + 1) * DK, :]
                        .rearrange("j p (q c) -> p q j c", q=2),
                        headT[1][hp][:, h * 1024:(h + 1) * 1024]
                        .rearrange("p (q j c) -> p q j c", q=2, j=NC))
                nc.gpsimd.collective_compute(
                    "AllToAll", mybir.AluOpType.bypass,
                    replica_groups=[list(range(NC))],
                    ins=[a2a1_in[h].opt()], outs=[a2a1_out[h].opt()])

            def fetch_rhs(n):
                # per-jj rhs loads so the first matmuls start as soon as the
                # first 32KB chunk lands (matters after the last collective)
                if n == 0:
                    src, ncols, tagn = a2a_out[0], CB, "r0"
                else:
                    src, ncols, tagn = a2a1_out[n - 1], 128, f"r1{n-1}"
                rhs_t = wos.tile([128, NC, CB], bf16, tag="rhs", name=f"rhs{tagn}")
                for jj in range(8):
                    nc.sync.dma_start(rhs_t[:, jj:jj + 1, :ncols],
                                      src[jj:jj + 1, :, :ncols].transpose([1, 0, 2]))
                return rhs_t

            def emit_wo(n, rhs_t=None):
                if n == 0:
                    ncols, col0, tagn = CB, 0, "r0"
                else:
                    h = n - 1
                    ncols, col0, tagn = 128, CB + h * 128, f"r1{h}"
                if rhs_t is None:
                    rhs_t = fetch_rhs(n)
                flush_pending_a()
                fl = False
                groups = ((0, 1, 2), (3, 4, 5), (6,), (7,)) if n == 2 \
                    else ((0, 1, 2), (3, 4, 5), (6, 7))
                for mts in groups:
                    pss = [sp.tile([128, 1024], f32, tag="sp", name=f"wp{tagn}{mt}")
                           for mt in mts]
                    for jj in range(8):
                        for k, mt in enumerate(mts):
                            nc.tensor.matmul(pss[k][:, :ncols],
                                             wo_sb[:, jj, mt * KT:(mt + 1) * KT],
                                             rhs_t[:, jj, :ncols],
                                             start=(jj == 0), stop=(jj == 7))
                    if not fl:
                        flush_pending()
                        fl = True
                    for k, mt in enumerate(mts):
                        ob = osb.tile([128, CB], f32, tag="osb", name=f"ob{tagn}{mt}")
                        nc.vector.tensor_scalar_add(ob[:, :ncols], pss[k][:, :ncols],
                                                    bo_sb[:, mt, :])
                        # scalar ring: exp is finished by the (tail-only) WO
                        # phase, and this keeps the sync ring free for the
                        # collective staging + rhs chunk reads
                        nc.scalar.dma_start(out_t[mt * KT:(mt + 1) * KT, col0:col0 + ncols],
                                            ob[:, :ncols])

            # ---- pipeline: attention interleaved between projection blocks;
            # WO chunks emitted as soon as their collective lands ----
            emit_inputs()
            nc.gpsimd.collective_compute(
                "AllReduce", mybir.AluOpType.add,
                replica_groups=[list(range(NC))],
                ins=[bar_in.opt()], outs=[bar_out.opt()])
            # projection blocks emitted in PAIRS (10-13us of dense matmuls,
            # so the second block runs at the fully-ramped clock) separated
            # by runs of attention blocks (exp-paced on ScalarE regardless
            # of PE clock)
            emit_proj_kv(0, 0)
            emit_proj_kv(0, 1)
            emit_proj_q(0, 0)
            emit_proj_q(0, 1)
            emit_attn_qs(0, 0)
            emit_proj_block(0, 2)      # flushes finish(0,0)
            emit_proj_block(0, 3)
            emit_attn_qs(0, 1)
            emit_attn_qs(0, 2)         # flushes finish(0,1)
            emit_proj_block(1, 0)      # flushes finish(0,2)
            emit_proj_block(1, 1)
            emit_attn_qs(0, 3)
            emit_attn_qs(1, 0)         # flushes finish(0,3)
            emit_a2a(0)
            emit_proj_block(1, 2)      # flushes finish(1,0)
            emit_proj_block(1, 3)
            emit_attn_qs(1, 1)
            flush_pending()            # finish(1,1) before its collective
            emit_a2a1_half(0)
            emit_attn_qs(1, 2)
            rhs0 = fetch_rhs(0)        # prefetch: sync ring is idle here and
                                       # a2a(0) completed long ago, so wo(0)
                                       # can start the instant it's reached
            emit_attn_qs(1, 3)         # flushes finish(1,2) internally
            # finish(1,3): den copies first, a few dependency-free matmuls
            # bridge the copy latency so the PE clock stays hot, then the
            # broadcast+normalize chain and the final collective's staging
            flush_pending_a()
            hot = [sp.tile([128, 1024], f32, tag="sp", name=f"hot{i}")
                   for i in range(2)]
            for i in range(7):
                nc.tensor.matmul(hot[i % 2][:, :SW], warm[:, :128],
                                 warm[:, :SW], start=True, stop=True)
            flush_pending()
            emit_a2a1_half(1)
            # wo(0) and wo(1) are deferred to AFTER the final collective's
            # staging: ~20us of collective-independent PE work that absorbs
            # peer launch skew while a2a1h1 completes
            emit_wo(0, rhs0)
            rhs1 = fetch_rhs(1)
            emit_wo(1, rhs1)
            # small hot-keeper bridge: under launch skew the final collective
            # outlasts wo0+wo1, and a cold PE would run wo(2) at half clock
            for i in range(10):
                nc.tensor.matmul(hot[i % 2][:, :SW], warm[:, :128],
                                 warm[:, :SW], start=True, stop=True)
            emit_wo(2)

    nc.compile()
    return nc


def kernel(x, y, mask, Wq, bq, Wk, bk, Wv, bv, Wo, bo, _trace=False):
    x = np.asarray(x, np.float32)
    y = np.asarray(y, np.float32)
    cls_, span_, pidx, pats = _classify_blocks(mask)

    key = (x.shape,
           tuple(tuple(c) for c in cls_),
           tuple(tuple(s) for s in span_),
           tuple(tuple(p) for p in pidx),
           pats.tobytes())
    if key not in _CACHE:
        _CACHE[key] = _build(cls_, span_, pidx, pats.shape[0])
    nc = _CACHE[key]

    fac = np.float32(1.0 / np.sqrt(DK))
    xtb = np.ascontiguousarray(
        np.concatenate([x[n].T for n in range(NB)], axis=1)).astype(BF16)
    ytb = np.ascontiguousarray(
        np.concatenate([y[n].T for n in range(NB)], axis=1)).astype(BF16)
    Wq32 = np.asarray(Wq, np.float32) * fac
    bq32 = np.asarray(bq, np.float32) * fac

    def pmajor(w):
        # [DM, X] -> [128, 8, X] with [p, t, :] = w[t*128+p, :]
        w = np.asarray(w)
        return np.ascontiguousarray(w.reshape(8, 128, w.shape[1]).transpose(1, 0, 2))

    wo_pm = pmajor(np.asarray(Wo, np.float32)).astype(BF16)
    bo_pm = pmajor(np.asarray(bo, np.float32).reshape(DM, 1))
    mpat_t = np.ascontiguousarray(pats.transpose(1, 0, 2))

    in_maps = []
    for c in range(NC):
        d0 = c * DPC
        wv_aug = np.zeros((DM, VW), np.float32)
        bv1 = np.zeros((1, VW), np.float32)
        for hp in range(HPC):
            h = HPC * c + hp
            wv_aug[:, hp * 65:hp * 65 + DK] = np.asarray(Wv, np.float32)[:, h * DK:(h + 1) * DK]
            bv1[0, hp * 65:hp * 65 + DK] = np.asarray(bv, np.float32)[h * DK:(h + 1) * DK]
            bv1[0, hp * 65 + DK] = 1.0
        in_maps.append({
            "xtb": xtb, "ytb": ytb,
            "wq": pmajor(Wq32[:, d0:d0 + DPC]).astype(BF16),
            "wk": pmajor(np.asarray(Wk, np.float32)[:, d0:d0 + DPC]).astype(BF16),
            "wv": pmajor(wv_aug).astype(BF16),
            "wo": wo_pm,
            "bq": bq32[d0:d0 + DPC].reshape(DPC, 1),
            "bk": np.asarray(bk, np.float32)[d0:d0 + DPC].reshape(DPC, 1),
            "bv1": bv1.astype(BF16),
            "bo": bo_pm,
            "mpat": mpat_t,
        })

    res = run_bass_kernel_spmd(nc, in_maps, core_ids=list(range(NC)), trace=_trace)
    out = np.empty((NB, L, DM), np.float32)
    for c in range(NC):
        ot = res.results[c]["out_t"]
        out[0, c * CB:(c + 1) * CB, :] = ot[:, :CB].T
        # batch 1: interleaved col->core map, 64 rows per (half, sub-window)
        for h in range(2):
            for q in range(2):
                r0 = (2 * h + q) * SW + c * 64
                out[1, r0:r0 + 64, :] = ot[:, CB + h * 128 + q * 64:
                                           CB + h * 128 + (q + 1) * 64].T
    if _trace:
        kernel.last_results = res
    return out
